# revision 46
# baseline (speedup 1.0000x reference)
"""MultiHeadCrossAttention Trainium2 kernel (8 NeuronCores, SPMD).

Problem: B=4, SQ=SK=2048, D=1024, H=16 (HD=64), f32 in/out.

Distribution (3 SPMD launches):
  Phase 1 (row-parallel): QKV projections in fp8 e4m3 with DoubleRow matmuls
    (2 contraction rows per partition -> 0.5 cyc/row). Weights host-prescaled
    by 64; outputs written as scaled fp8 (q*4, k*2, v*1).
  Phase 2 (head-parallel): attention, 2 heads/core. Keys mask-compacted on
    host. Scores S = (4q).(2k) accumulate in PSUM via fp8-DR matmuls over the
    hd=64 contraction split as [32 partitions x 2 slots]; 64*bias (fp8) is
    injected into the same PSUM via a DoubleRow identity matmul, so
    exp(score+bias) needs no elementwise multiply. exp runs split across
    engines: ScalarE true exp (scale=1/64) -> fp8 weights (DR AV matmul), and
    Schraudolph bit-trick exp on DVE/Pool (tensor_scalar -> int16, bitcast to
    bf16 -> plain AV matmul). Mask + normalizer ride as an extra fp8 value
    column; normalization multiplies by 32/norm -> fp8 ctx output.
  Phase 3 (row-parallel): out projection (fp8 DR, ctxT*32 @ woT*64, /2048
    folded into the residual add), one-pass mean/var LayerNorm.
"""

import sys

sys.path.insert(0, "/opt/trn_rl_repo")

import numpy as np
import ml_dtypes

import concourse.bass as bass
import concourse.tile as tile
from concourse import bacc, mybir
from concourse import bass_utils

BF16 = ml_dtypes.bfloat16

B, SQ, SK, D, H = 4, 2048, 2048, 1024, 16
HD = D // H  # 64
NCORES = 8
HPC = H // NCORES          # heads per core = 2
RPC = B * SQ // NCORES     # rows per core (phases 1/3) = 1024
LN_EPS = 1e-5

dt = mybir.dt
AF = mybir.ActivationFunctionType
ALU = mybir.AluOpType
MPM = mybir.MatmulPerfMode

F8 = np.dtype(mybir.dt.np(dt.float8e4))
F32 = np.float32

# Schraudolph fast-exp in fp8 e4m3 bit space: e4m3_bits(exp(x)) ~= x*8*log2e + B
# (saturating round-to-nearest float->int8: low saturation -> -128 = -0.0 in
# e4m3fn = dropped weight; high saturation unreachable, scores < 6.1)
SCH_A8 = 8.0 * 1.4426950408889634
SCH_B8 = 7.0 * 8.0 - 0.0436 * 8.0

_programs = {}


# --------------------------------------------------------------------------
# Phase 1: QKV projection (row-parallel, fp8 DoubleRow, no bias on device —
# host folds biases into the outputs if nonzero).
#   inputs (per core): xqT/xkT/xvT [D, RPC] fp8 (input^T), wqT/wkT/wvT
#                      [D, D] fp8 (W^T * 64)
#   outputs: qT_o/kT_o [D, RPC] fp8 (4*q^T, 2*k^T), v_o [RPC, D] fp8 (v)
# --------------------------------------------------------------------------
def build_phase1(reps=1):
    nc = bacc.Bacc("TRN2", debug=False, num_devices=NCORES)
    KC = D // 128  # 8 chunks of 128 = 4 double-chunks

    ins = {}
    for nm in ("xqT", "xkT", "xvT"):
        ins[nm] = nc.dram_tensor(nm, [D, RPC], dt.float8e4, kind="ExternalInput").ap()
    for nm in ("wqT", "wkT", "wvT"):
        ins[nm] = nc.dram_tensor(nm, [D, D], dt.float8e4, kind="ExternalInput").ap()
    qT_o = nc.dram_tensor("qT_o", [D, RPC], dt.float8e4, kind="ExternalOutput").ap()
    kT_o = nc.dram_tensor("kT_o", [D, RPC], dt.float8e4, kind="ExternalOutput").ap()
    v_o = nc.dram_tensor("v_o", [RPC, D], dt.float8e4, kind="ExternalOutput").ap()

    # greedy engine assignment for the 48 PSUM->SBUF scaled copies
    # (GPSIMD/Pool cannot touch PSUM on TRN2, so only Act/DVE)
    eng_cost = {"act": 570.0, "dve": 660.0}
    eng_load = {"act": 0.0, "dve": 0.0}
    copy_plan = []
    for _ in range(3 * (D // 128) * 2):
        e = min(eng_load, key=lambda k: eng_load[k] + eng_cost[k])
        copy_plan.append(e)
        eng_load[e] += eng_cost[e]
    copy_i = [0]

    with tile.TileContext(nc) as tc:
        with (
            tc.tile_pool(name="big", bufs=1) as bigp,
            tc.tile_pool(name="outp", bufs=3) as outp,
            tc.tile_pool(name="ps", bufs=2, space="PSUM") as psp,
        ):
            warm = bigp.tile([1, 1], dt.float32)
            nc.vector.memset(warm[:], 1.0)
            warm2 = bigp.tile([1, 1], dt.float32)
            nc.scalar.activation(warm2[:], warm[:], AF.Copy)
            sb = {}
            for nm in ("xqT", "xkT", "xvT", "wqT", "wkT", "wvT"):
                ncols = ins[nm].shape[1]
                sb[nm] = bigp.tile([128, KC, ncols], dt.float8e4, name=f"{nm}_sb")
            def load_part(nm, k0, k1):
                nc.sync.dma_start(
                    sb[nm][:, k0:k1, :],
                    ins[nm][128 * k0 : 128 * k1, :].rearrange(
                        "(k p) c -> p k c", p=128
                    ),
                )

            def load_full(nm):
                nc.sync.dma_start(
                    sb[nm][:], ins[nm][:, :].rearrange("(k p) c -> p k c", p=128)
                )

            def copy_out(dst, src, scale):
                e = copy_plan[copy_i[0] % len(copy_plan)]
                copy_i[0] += 1
                if e == "act":
                    nc.scalar.activation(dst, src, AF.Copy, scale=scale)
                elif e == "dve":
                    nc.vector.tensor_scalar(
                        out=dst, in0=src, scalar1=scale, scalar2=None, op0=ALU.mult
                    )
                else:
                    nc.gpsimd.tensor_scalar(
                        out=dst, in0=src, scalar1=scale, scalar2=None, op0=ALU.mult
                    )

            def proj(x_nm, w_nm, out_dram, transposed_out, scale):
                xt = sb[x_nm]
                wt = sb[w_nm]
                if transposed_out:
                    lt, rt = wt, xt   # out[d_out, rows]
                else:
                    lt, rt = xt, wt   # out[rows, d_out]
                n_m = lt.shape[2] // 128
                n_n = rt.shape[2] // 512
                MG = 2
                for mg in range(0, n_m, MG):
                    ms = range(mg, min(mg + MG, n_m))
                    pss = {}
                    for m in ms:
                        for n in range(n_n):
                            pss[m, n] = psp.tile(
                                [128, 512], dt.float32, name="ps", tag=f"ps{m % MG}_{n}"
                            )
                    for k2 in range(KC // 2):
                        for m in ms:
                            for n in range(n_n):
                                nc.tensor.matmul(
                                    pss[m, n][:],
                                    lhsT=lt[:, 2 * k2 : 2 * k2 + 2, m * 128 : (m + 1) * 128],
                                    rhs=rt[:, 2 * k2 : 2 * k2 + 2, n * 512 : (n + 1) * 512],
                                    start=(k2 == 0),
                                    stop=(k2 == KC // 2 - 1),
                                    perf_mode=MPM.DoubleRow,
                                )
                    osb = outp.tile(
                        [128, MG, rt.shape[2]], dt.float8e4, name=f"{x_nm}_osb", tag="osb"
                    )
                    for m in ms:
                        for n in range(n_n):
                            copy_out(osb[:, m - mg, n * 512 : (n + 1) * 512], pss[m, n][:], scale)
                    nc.sync.dma_start(
                        out_dram[mg * 128 : (mg + MG) * 128, :].rearrange(
                            "(g p) c -> p g c", p=128
                        ),
                        osb[:],
                    )

            for r in range(reps):
                # just-in-time load emission: each projection's inputs get
                # DMA queue slots only once the previous projection's
                # output traffic is queued, keeping the (serial) DMA pipe
                # in consumption order
                if r == 0:
                    for k0, k1 in ((0, 2), (2, 4), (4, 8)):
                        load_part("wqT", k0, k1)
                        load_part("xqT", k0, k1)
                    for k0, k1 in ((0, 4), (4, 8)):
                        load_part("wkT", k0, k1)
                        load_part("xkT", k0, k1)
                proj("xqT", "wqT", qT_o, True, 4.0 / 64.0)
                if r == 0:
                    for k0, k1 in ((0, 4), (4, 8)):
                        load_part("wvT", k0, k1)
                        load_part("xvT", k0, k1)
                proj("xkT", "wkT", kT_o, True, 2.0 / 64.0)
                proj("xvT", "wvT", v_o, False, 1.0 / 64.0)

    nc.compile()
    return nc


# --------------------------------------------------------------------------
# Phase 2: attention (head-parallel, 2 heads/core).
#   inputs (per core):
#     qT  [128, B*SQ] fp8  (rows = 2 heads x 64 dims, = 4*q^T)
#     kT  [128, TNV] fp8   (compacted, = 2*k^T)
#     va  [TNV, HPC*(HD+1)] fp8 (v*mask | mask column per head)
#     eb  [HPC, TNV, SQ] fp8 (64 * bias^T per head, compacted rows)
#     i2  [128, 256] fp8   (DoubleRow identity: [:, :128]=I, [:, 128:]=0)
#   outputs: ctx_o [128, B*SQ] fp8 = 32*ctx/norm in [p, t, d] layout
#
# Per (qc, b) iteration both heads' score tiles live in one 2-bank PSUM
# pair [128, 2, 512]; a single 1024-col op drains the pair to fp8 weights:
# ScalarE true exp (scale=1/64) or DVE e4m3-Schraudolph (affine -> int8 bit
# pattern, saturating round-to-nearest; bits = 8*log2e*score + 55.65, low
# saturation hits -128 = -0.0 = dropped weight). Both paths feed the same
# fp8 DoubleRow AV matmuls. Normalization: one bf16 copy of ctx PSUM per
# ti (Act/DVE), then per-(tt,h) divides by the norm column on the
# otherwise-idle GPSIMD engine.
# --------------------------------------------------------------------------
def build_phase2(nvts=(8, 8, 8, 8), reps=1, sp_bufs=3, cp_bufs=2, wm_bufs=4,
                 av_lag=3, plan_start_act=False, norm_base=3):
    nc = bacc.Bacc("TRN2", debug=False, num_devices=NCORES)
    QC = 512
    NQC = SQ // QC
    snvt = [0]
    for t in nvts:
        snvt.append(snvt[-1] + t)
    TNT = snvt[-1]
    TNV = TNT * 128
    NTMAX = max(nvts)

    qT = nc.dram_tensor("qT", [128, B * SQ], dt.float8e4, kind="ExternalInput").ap()
    kT = nc.dram_tensor("kT", [128, TNV], dt.float8e4, kind="ExternalInput").ap()
    va = nc.dram_tensor("va", [TNV, HPC * (HD + 1)], dt.float8e4, kind="ExternalInput").ap()
    # bias pre-slabbed per iteration: [128, kj, h, q] blocks, so each slab
    # is one contiguous DMA issue (HWDGE slots are the ramp-in bottleneck)
    ebs = nc.dram_tensor(
        "ebs", [128, NQC * TNT * HPC * QC], dt.float8e4, kind="ExternalInput"
    ).ap()
    i2 = nc.dram_tensor("i2", [128, 256], dt.float8e4, kind="ExternalInput").ap()
    ctx_o = nc.dram_tensor("ctx_o", [128, B * SQ], dt.float8e4, kind="ExternalOutput").ap()

    # per-iteration drain engine plan: balance act (1038ns/pair drain) vs
    # dve (1192ns/pair drain), with the 2 norm copies (~790ns) placed on
    # whichever engine leaves the lower makespan for this NT.
    def drain_plan(NT):
        best = None
        for copies_on_act in (True,):
            ca = 720.0 if copies_on_act else 0.0
            cd = 0.0 if copies_on_act else 720.0
            for n in range(NT + 1):
                m = max(n * 1038.0 + ca, (NT - n) * 1192.0 + cd)
                if best is None or m < best[0]:
                    best = (m, n, copies_on_act)
        _, n_act, copies_on_act = best
        n_dve = NT - n_act
        plan = []
        a = d = 0
        par = 0 if plan_start_act else 1
        for kj in range(NT):
            # alternate; exhaust the smaller quota gracefully
            if (kj % 2 == par and a < n_act) or d >= n_dve:
                plan.append("act")
                a += 1
            else:
                plan.append("dve")
                d += 1
        return plan, copies_on_act

    with tile.TileContext(nc) as tc:
        with (
            tc.tile_pool(name="big", bufs=1) as bigp,
            tc.tile_pool(name="ebp", bufs=5) as ebp,
            tc.tile_pool(name="wp", bufs=wm_bufs) as wp,
            tc.tile_pool(name="np_", bufs=4) as normp,
            tc.tile_pool(name="Sp", bufs=sp_bufs, space="PSUM") as Sp,
            tc.tile_pool(name="cp", bufs=cp_bufs, space="PSUM") as cp,
        ):
            # hd-split layouts for DoubleRow: [32 partitions, 2 slots, head, cols]
            qT_sb = bigp.tile([32, HPC, 2, B * SQ], dt.float8e4)
            kT_sb = bigp.tile([32, HPC, 2, TNV], dt.float8e4)
            va_sb = bigp.tile([128, TNT, HPC * (HD + 1)], dt.float8e4)
            i2_sb = bigp.tile([128, 2, 128], dt.float8e4)
            warm = bigp.tile([1, 1], dt.float32)
            nc.vector.memset(warm[:], 0.0)
            warm2 = bigp.tile([1, 1], dt.float32)
            nc.scalar.activation(warm2[:], warm[:], AF.Exp)

            def load_qk(b):
                # one DMA per tensor per batch (both heads at once) — DMA
                # issue slots on HWDGE are ~630ns each and serialize, so
                # fewer, bigger issues matter at ramp-in
                nc.sync.dma_start(
                    qT_sb[:, :, :, b * SQ : (b + 1) * SQ],
                    qT[:, b * SQ : (b + 1) * SQ].rearrange(
                        "(h s p) c -> p h s c", p=32, s=2
                    ),
                )
                cs, ce = snvt[b] * 128, snvt[b + 1] * 128
                nc.sync.dma_start(
                    kT_sb[:, :, :, cs:ce],
                    kT[:, cs:ce].rearrange("(h s p) c -> p h s c", p=32, s=2),
                )

            def load_va(b):
                cs, ce = snvt[b] * 128, snvt[b + 1] * 128
                nc.sync.dma_start(
                    va_sb[:, snvt[b] : snvt[b + 1], :],
                    va[cs:ce, :].rearrange("(t p) d -> p t d", p=128),
                )

            def load_b(b):
                load_qk(b)
                load_va(b)

            load_qk(0)
            nc.scalar.dma_start(i2_sb[:], i2[:])

            iters = [(qc, b) for qc in range(NQC) for b in range(B)] * reps

            # per-iteration column offsets into ebs
            it_off = []
            off = 0
            for qc_ in range(NQC):
                for b_ in range(B):
                    it_off.append(off)
                    off += nvts[b_] * HPC * QC

            def load_slab(it_i, head_kjs=None, eng=None):
                qc_, b_ = iters[it_i % (NQC * B)]
                NT = nvts[b_]
                eng = eng or nc.gpsimd
                eb_sb = ebp.tile(
                    [128, NTMAX + 1, HPC, QC], dt.float8e4, name="eb_sb", tag="eb"
                )
                o = it_off[it_i % (NQC * B)]
                W = HPC * QC
                if head_kjs:
                    eng.dma_start(
                        eb_sb[:, 0:head_kjs, :, :], ebs[:, o : o + head_kjs * W]
                    )

                    def rest(reng=None):
                        reng = reng or nc.gpsimd
                        if head_kjs < NT:
                            reng.dma_start(
                                eb_sb[:, head_kjs:NT, :, :],
                                ebs[:, o + head_kjs * W : o + NT * W],
                            )
                        nc.gpsimd.memset(eb_sb[:, NT, :, :], 0.0)

                    return eb_sb, rest
                eng.dma_start(eb_sb[:, 0:NT, :, :], ebs[:, o : o + NT * W])
                # pad tile (read by the DR inject's zero slot on the last key
                # tile) must be initialized for the race detector
                nc.gpsimd.memset(eb_sb[:, NT, :, :], 0.0)
                return eb_sb

            slabs = {}
            # first two key tiles of iteration 0 land before the va bulk
            # loads (first inject reads the kj 0-1 DR pair); the slab-0
            # tail waits until the critical b0/b1 loads have their DMA
            # queue slots (SWDGE bulk transfers would cut the line)
            slabs[0], slab0_rest = load_slab(0, head_kjs=2, eng=nc.scalar)
            load_va(0)
            load_b(1)
            # early slab traffic must take HWDGE slots (scalar queue): the
            # SWDGE path would win the DMA-engine race against the critical
            # qT/kT/va loads above
            slab0_rest()
            slabs[1] = load_slab(1)
            for b in range(2, B):
                load_b(b)
                slabs[b] = load_slab(b)

            def emit_norm_ti(state, ti):
                # One bf16 copy of the ctx pair tile (frees the PSUM bank),
                # then per-(tt, h) divides by the norm column on GPSIMD.
                # The mask column is 1/32, so ctx/normcol = 32*ctx/sum(w):
                # the x32 ctx scaling is free.
                ctx, col0, holder = state[0], state[1], state[2]
                if holder[0] is None:
                    holder[0] = normp.tile(
                        [128, QC // 128, HPC * HD], dt.float8e4, name="ctxn", tag="ctxn"
                    )
                ctxn = holder[0]
                ctxc = normp.tile(
                    [128, 2, HPC * (HD + 1)], dt.float32, name="ctxc", tag=f"ctxc{ti}"
                )
                if state[3]:
                    nc.scalar.activation(ctxc[:], ctx[ti][:], AF.Copy)
                else:
                    nc.vector.tensor_copy(out=ctxc[:], in_=ctx[ti][:])
                # Pool only supports the mult tensor_scalar form: one small
                # strided reciprocal on DVE (from SBUF), then Pool multiplies
                rec = normp.tile([128, 2, HPC], dt.float32, name="rec", tag=f"rec{ti}")
                nc.vector.reciprocal(rec[:], ctxc[:, :, HD :: HD + 1])
                for tt in range(2):
                    t = ti * 2 + tt
                    for h in range(HPC):
                        nc.gpsimd.tensor_scalar(
                            out=ctxn[:, t, h * HD : (h + 1) * HD],
                            in0=ctxc[:, tt, h * (HD + 1) : h * (HD + 1) + HD],
                            scalar1=rec[:, tt, h : h + 1],
                            scalar2=None,
                            op0=ALU.mult,
                        )
                if ti == QC // 256 - 1:
                    nc.sync.dma_start(ctx_o[:, col0 : col0 + QC], ctxn[:])

            def emit_norm(state):
                while state[2][1] < QC // 256:
                    emit_norm_ti(state, state[2][1])
                    state[2][1] += 1

            def emit_av_pair(ctx, tbase, pj, wm2, start, stop):
                # DoubleRow fp8 AV over a kj pair (weights from either exp path)
                for ti in range(QC // 256):
                    for tt in range(2):
                        for h in range(HPC):
                            t = ti * 2 + tt
                            nc.tensor.matmul(
                                ctx[ti][:, tt, h * (HD + 1) : (h + 1) * (HD + 1)],
                                lhsT=wm2[:, :, h, t * 128 : (t + 1) * 128].bitcast(
                                    dt.float8e4
                                ),
                                rhs=va_sb[:, tbase + 2 * pj : tbase + 2 * pj + 2,
                                          h * (HD + 1) : (h + 1) * (HD + 1)],
                                start=start and (tt == 0) and (h == 0),
                                stop=stop and (ti == QC // 256 - 1) and (tt == 1) and (h == HPC - 1),
                                perf_mode=MPM.DoubleRow,
                                skip_group_check=True,
                            )

            def emit_av_one(ctx, tbase, kj, wm1, sl, start, stop):
                # plain fp8 AV for the odd tail tile
                for ti in range(QC // 256):
                    for tt in range(2):
                        for h in range(HPC):
                            t = ti * 2 + tt
                            nc.tensor.matmul(
                                ctx[ti][:, tt, h * (HD + 1) : (h + 1) * (HD + 1)],
                                lhsT=wm1[:, sl, h, t * 128 : (t + 1) * 128].bitcast(
                                    dt.float8e4
                                ),
                                rhs=va_sb[:, tbase + kj, h * (HD + 1) : (h + 1) * (HD + 1)],
                                start=start and (tt == 0) and (h == 0),
                                stop=stop and (ti == QC // 256 - 1) and (tt == 1) and (h == HPC - 1),
                                skip_group_check=True,
                            )

            # AV matmuls are deferred AV_LAG kj steps after their weights'
            # drain is issued, so by fetch time the drain semaphore is
            # already satisfied and they never park in PE's 4-deep wait
            # queue (a full wait queue blocks PE.SEQ instruction fetch).
            AV_LAG = av_lag
            NB = AV_LAG if norm_base is None else norm_base
            av_q = []        # (ready_gkj, fn, args, stop)
            tail_norm = None
            gkj = [0]        # global kj counter across iterations

            def flush_av(g):
                while av_q and av_q[0][0] <= g:
                    _, fn, args, stp = av_q.pop(0)
                    fn(*args, stop=stp)

            for it_i, (qc, b) in enumerate(iters):
                NT = nvts[b]
                eb_sb = slabs.pop(it_i)
                if it_i + 4 < len(iters):
                    slabs[it_i + 4] = load_slab(it_i + 4)
                ctx = [
                    cp.tile([128, 2, HPC * (HD + 1)], dt.float32, name=f"ctx{t}", tag="ctx")
                    for t in range(QC // 256)
                ]
                col0 = b * SQ + qc * QC
                tbase = snvt[b]
                plan, copies_on_act = drain_plan(NT)
                if tail_norm is not None:
                    tail_norm[3] = copies_on_act

                def make_S(kj):
                    # both heads' scores in one 2-bank PSUM pair tile; the
                    # bias injects stay per-head ([128, 512] dst — a matmul
                    # dst may not cross a PSUM bank)
                    kcol = tbase * 128 + kj * 128
                    S = Sp.tile([128, 2, QC], dt.float32, name="S", tag="S")
                    for h in range(HPC):
                        nc.tensor.matmul(
                            S[:, h, :],
                            lhsT=kT_sb[:, h, :, kcol : kcol + 128],
                            rhs=qT_sb[:, h, :, col0 : col0 + QC],
                            start=True,
                            stop=False,
                            perf_mode=MPM.DoubleRow,
                            skip_group_check=True,
                        )
                        nc.tensor.matmul(
                            S[:, h, :],
                            lhsT=i2_sb[:],
                            rhs=eb_sb[:, kj : kj + 2, h, :],
                            start=False,
                            stop=True,
                            perf_mode=MPM.DoubleRow,
                            skip_group_check=True,
                        )
                    return S

                first_av = [True]
                wm_cur = [None]
                for kj in range(NT):
                    g = gkj[0] + kj
                    S = make_S(kj)
                    # AV flush first: the previous iteration's tail AVs
                    # (ready at local kj <= AV_LAG-1) must be emitted before
                    # its norm below reads those ctx banks; this iteration's
                    # own first AV flush (ready at kj AV_LAG+1 > NB+1) then
                    # lands after both norm tis
                    flush_av(g)
                    if tail_norm is not None and NB <= kj <= NB + 1:
                        emit_norm_ti(tail_norm, kj - NB)
                        tail_norm[2][1] += 1
                        if tail_norm[2][1] >= QC // 256:
                            tail_norm = None
                    # exp drain of the [128, 1024] pair: ScalarE true exp or
                    # DVE e4m3 Schraudolph, per the balance plan
                    sl = kj % 2
                    if sl == 0:
                        wm_cur[0] = wp.tile(
                            [128, 2, HPC, QC], dt.int8, name="wm", tag="wm"
                        )
                    wm = wm_cur[0]
                    if plan[kj] == "act":
                        nc.scalar.activation(
                            wm[:, sl].bitcast(dt.float8e4), S[:], AF.Exp,
                            scale=1.0 / 64.0,
                        )
                    elif plan[kj] == "dve":
                        nc.vector.tensor_scalar(
                            out=wm[:, sl], in0=S[:], scalar1=SCH_A8 / 64.0,
                            scalar2=SCH_B8, op0=ALU.mult, op1=ALU.add,
                        )
                    else:  # mix: one head per engine
                        nc.scalar.activation(
                            wm[:, sl, 0].bitcast(dt.float8e4), S[:, 0, :],
                            AF.Exp, scale=1.0 / 64.0,
                        )
                        nc.vector.tensor_scalar(
                            out=wm[:, sl, 1], in0=S[:, 1, :],
                            scalar1=SCH_A8 / 64.0, scalar2=SCH_B8,
                            op0=ALU.mult, op1=ALU.add,
                        )
                    last = kj == NT - 1
                    if sl == 1:
                        av_q.append(
                            (g + AV_LAG, emit_av_pair,
                             [ctx, tbase, kj // 2, wm, first_av[0]], last)
                        )
                        first_av[0] = False
                    elif last:
                        # odd tail tile
                        av_q.append(
                            (g + AV_LAG, emit_av_one,
                             [ctx, tbase, kj, wm, sl, first_av[0]], True)
                        )
                        first_av[0] = False

                if tail_norm is not None:
                    # short iterations: flush the PREVIOUS iteration's AVs
                    # (ready <= gkj+AV_LAG), then its norm, before rotating
                    flush_av(gkj[0] + AV_LAG)
                    emit_norm(tail_norm)
                gkj[0] += NT
                tail_norm = [ctx, col0, [None, 0], True]

            flush_av(10 ** 9)
            emit_norm(tail_norm)

    nc.compile()
    return nc


# --------------------------------------------------------------------------
# Phase 3: out projection + residual + LayerNorm (row-parallel, fp8 DR GEMM).
#   inputs (per core): ctxT [D, RPC] fp8 (=32*ctx^T), woT [D, D] fp8 (=64*Wo^T),
#     resid [RPC, D] bf16 (query rows + bo), [gammab/betab [128, D] f32 if
#     not trivial_ln]
#   outputs: out_o [RPC, D] f32
# --------------------------------------------------------------------------
def build_phase3(trivial_ln=True, reps=1):
    nc = bacc.Bacc("TRN2", debug=False, num_devices=NCORES)
    KC = D // 128

    ctxT = nc.dram_tensor("ctxT", [D, RPC], dt.float8e4, kind="ExternalInput").ap()
    woT = nc.dram_tensor("woT", [D, D], dt.float8e4, kind="ExternalInput").ap()
    resid = nc.dram_tensor("resid", [RPC, D], dt.bfloat16, kind="ExternalInput").ap()
    if not trivial_ln:
        gammab = nc.dram_tensor("gammab", [128, D], dt.float32, kind="ExternalInput").ap()
        betab = nc.dram_tensor("betab", [128, D], dt.float32, kind="ExternalInput").ap()
    out_o = nc.dram_tensor("out_o", [RPC, D], dt.bfloat16, kind="ExternalOutput").ap()
    PS_SCALE = 1.0 / (32.0 * 64.0)

    with tile.TileContext(nc) as tc:
        with (
            tc.tile_pool(name="big", bufs=1) as bigp,
            tc.tile_pool(name="rp", bufs=4) as rp,
            tc.tile_pool(name="wk", bufs=3) as wk,
            tc.tile_pool(name="ps", bufs=3, space="PSUM") as psp,
        ):
            ctx_sb = bigp.tile([128, KC, RPC], dt.float8e4)
            wo_sb = bigp.tile([128, KC, D], dt.float8e4)
            for c in range(KC // 2):
                nc.sync.dma_start(
                    ctx_sb[:, 2 * c : 2 * c + 2, :],
                    ctxT[256 * c : 256 * (c + 1), :].rearrange(
                        "(k p) c -> p k c", p=128
                    ),
                )
                nc.sync.dma_start(
                    wo_sb[:, 2 * c : 2 * c + 2, :],
                    woT[256 * c : 256 * (c + 1), :].rearrange(
                        "(k p) c -> p k c", p=128
                    ),
                )
            eps_sb = bigp.tile([128, 1], dt.float32)
            nc.vector.memset(eps_sb[:], LN_EPS)
            warm = bigp.tile([1, 1], dt.float32)
            nc.vector.memset(warm[:], 1.0)
            warm2 = bigp.tile([1, 1], dt.float32)
            nc.scalar.activation(warm2[:], warm[:], AF.Sqrt)
            warm3 = bigp.tile([1, 1], dt.float32)
            nc.scalar.activation(warm3[:], warm[:], AF.Square)
            if not trivial_ln:
                gam_sb = bigp.tile([128, D], dt.float32)
                nc.sync.dma_start(gam_sb[:], gammab[:])
                bet_sb = bigp.tile([128, D], dt.float32)
                nc.sync.dma_start(bet_sb[:], betab[:])

            for m in [m for _ in range(reps) for m in range(RPC // 128)]:
                res_sb = rp.tile([128, D], dt.bfloat16, name="res_sb", tag="res")
                nc.sync.dma_start(res_sb[:], resid[m * 128 : (m + 1) * 128, :])
                ps = psp.tile([128, 2, 512], dt.float32, name="ps", tag="ps")
                for n in range(2):
                    for k2 in range(KC // 2):
                        nc.tensor.matmul(
                            ps[:, n, :],
                            lhsT=ctx_sb[:, 2 * k2 : 2 * k2 + 2, m * 128 : (m + 1) * 128],
                            rhs=wo_sb[:, 2 * k2 : 2 * k2 + 2, n * 512 : (n + 1) * 512],
                            start=(k2 == 0),
                            stop=(k2 == KC // 2 - 1),
                            perf_mode=MPM.DoubleRow,
                        )
                x_sb = wk.tile([128, D], dt.float32, name="x_sb", tag="x")
                acc = wk.tile([128, 1], dt.float32, name="acc", tag="acc")
                # one 1024-wide drain of the 2-bank GEMM psum, residual add
                # and row-sum accumulation fused
                nc.vector.scalar_tensor_tensor(
                    out=x_sb[:],
                    in0=ps[:],
                    scalar=PS_SCALE,
                    in1=res_sb[:],
                    op0=ALU.mult,
                    op1=ALU.add,
                    accum_out=acc[:],
                )
                # the small LN chain rides the idle GPSIMD engine
                mu = wk.tile([128, 1], dt.float32, name="mu", tag="mu")
                nc.vector.tensor_scalar(
                    out=mu[:], in0=acc[:], scalar1=1.0 / D, scalar2=None,
                    op0=ALU.mult,
                )
                sq = wk.tile([128, D], dt.bfloat16, name="sq", tag="sq")
                s2 = wk.tile([128, 1], dt.float32, name="s2", tag="s2")
                nc.scalar.activation(sq[:], x_sb[:], AF.Square, accum_out=s2[:])
                var = wk.tile([128, 1], dt.float32, name="var", tag="var")
                # var = s2/D - mu^2  (one fused op: (s2*(1/D)) - mu2)
                mu2 = wk.tile([128, 1], dt.float32, name="mu2", tag="mu2")
                nc.vector.tensor_tensor(mu2[:], mu[:], mu[:], op=ALU.mult)
                nc.vector.tensor_scalar(
                    out=var[:], in0=s2[:], scalar1=1.0 / D, scalar2=mu2[:],
                    op0=ALU.mult, op1=ALU.subtract,
                )
                std = wk.tile([128, 1], dt.float32, name="std", tag="std")
                nc.scalar.activation(std[:], var[:], AF.Sqrt, bias=eps_sb[:])
                rstd = wk.tile([128, 1], dt.float32, name="rstd", tag="rstd")
                nc.vector.reciprocal(rstd[:], std[:])
                # mrsn = -mu*rstd, usable as both DVE scalar2 (op1=add) and
                # Act activation bias
                mrsn = wk.tile([128, 1], dt.float32, name="mrsn", tag="mrsn")
                nc.vector.tensor_scalar(
                    out=mrsn[:], in0=rstd[:], scalar1=mu[:, 0:1], scalar2=-1.0,
                    op0=ALU.mult, op1=ALU.mult,
                )
                out_sb = wk.tile([128, D], dt.bfloat16, name="out_sb", tag="out_sb")
                if trivial_ln:
                    nc.vector.tensor_scalar(
                        out=out_sb[:], in0=x_sb[:], scalar1=rstd[:], scalar2=mrsn[:],
                        op0=ALU.mult, op1=ALU.add,
                    )
                else:
                    tmp = wk.tile([128, D], dt.float32, name="tmp", tag="tmp")
                    nc.vector.tensor_scalar(
                        out=tmp[:], in0=x_sb[:], scalar1=rstd[:], scalar2=mrsn[:],
                        op0=ALU.mult, op1=ALU.add,
                    )
                    y = wk.tile([128, D], dt.float32, name="y", tag="y")
                    nc.vector.scalar_tensor_tensor(
                        out=y[:], in0=tmp[:], scalar=0.0, in1=gam_sb[:],
                        op0=ALU.add, op1=ALU.mult,
                    )
                    nc.gpsimd.tensor_add(out_sb[:], y[:], bet_sb[:])
                nc.sync.dma_start(out_o[m * 128 : (m + 1) * 128, :], out_sb[:])

    nc.compile()
    return nc


def _get_program(key, builder, *args, **kwargs):
    if key not in _programs:
        _programs[key] = builder(*args, **kwargs)
    return _programs[key]


def _run(nc, in_maps):
    return bass_utils.run_bass_kernel_spmd(nc, in_maps, core_ids=list(range(NCORES)))


def kernel(query, key, value, attention_mask, relative_position_bias,
           Wq, bq, Wk, bk, Wv, bv, Wo, bo, ln_gamma, ln_beta,
           _collect_results=None):
    query = np.asarray(query, dtype=np.float32)
    key = np.asarray(key, dtype=np.float32)
    value = np.asarray(value, dtype=np.float32)
    attention_mask = np.asarray(attention_mask)
    relative_position_bias = np.asarray(relative_position_bias, dtype=np.float32)

    def xT8(x):
        return np.ascontiguousarray(x.reshape(-1, D).T).astype(F8)

    def wT8(W, scale):
        return (np.ascontiguousarray(np.asarray(W, np.float32).T) * scale).astype(F8)

    xqT = xT8(query)
    xkT = xT8(key)
    xvT = xT8(value)
    wqT = wT8(Wq, 64.0)
    wkT = wT8(Wk, 64.0)
    wvT = wT8(Wv, 64.0)

    # ---------------- phase 1 ----------------
    in1 = []
    for c in range(NCORES):
        sl = slice(c * RPC, (c + 1) * RPC)
        in1.append({
            "xqT": np.ascontiguousarray(xqT[:, sl]),
            "xkT": np.ascontiguousarray(xkT[:, sl]),
            "xvT": np.ascontiguousarray(xvT[:, sl]),
            "wqT": wqT, "wkT": wkT, "wvT": wvT,
        })
    r1 = _run(_get_program("p1", build_phase1), in1)

    qT_full = np.empty((D, B * SQ), dtype=F8)
    kT_full = np.empty((D, B * SK), dtype=F8)
    v_full = np.empty((B * SK, D), dtype=F8)
    for c in range(NCORES):
        sl = slice(c * RPC, (c + 1) * RPC)
        qT_full[:, sl] = r1.results[c]["qT_o"]
        kT_full[:, sl] = r1.results[c]["kT_o"]
        v_full[sl, :] = r1.results[c]["v_o"]

    # fold any nonzero projection biases in on the host (zero in practice)
    if np.any(np.asarray(bq)):
        qT_full = (qT_full.astype(np.float32)
                   + 4.0 * np.asarray(bq, np.float32)[:, None]).astype(F8)
    if np.any(np.asarray(bk)):
        kT_full = (kT_full.astype(np.float32)
                   + 2.0 * np.asarray(bk, np.float32)[:, None]).astype(F8)
    if np.any(np.asarray(bv)):
        v_full = (v_full.astype(np.float32)
                  + np.asarray(bv, np.float32)[None, :]).astype(F8)

    # ---------------- phase 2 ----------------
    mask2 = (attention_mask.reshape(B, SK) != 0)
    valid = [np.nonzero(mask2[b])[0] for b in range(B)]
    nvts = tuple(max(1, -(-len(ix) // 128)) for ix in valid)
    snvt = np.concatenate([[0], np.cumsum(nvts)]).astype(int)
    TNT = int(snvt[-1])
    idx_pad = np.zeros(TNT * 128, dtype=np.int64)
    maskc = np.zeros((TNT * 128,), dtype=bool)
    for b in range(B):
        ix = valid[b]
        o = snvt[b] * 128
        idx_pad[o : o + len(ix)] = ix
        maskc[o : o + len(ix)] = True

    col_idx = (np.repeat(np.arange(B) * SK, np.array(nvts) * 128) + idx_pad)
    kT_c = np.ascontiguousarray(kT_full[:, col_idx])
    v_rows = v_full[col_idx, :]
    va_all = np.zeros((TNT * 128, H * (HD + 1)), dtype=F8)
    inv32 = np.asarray(1.0 / 32.0, dtype=F8)[()]
    for h in range(H):
        blk = np.where(maskc[:, None], v_rows[:, h * HD : (h + 1) * HD], np.zeros((), F8))
        va_all[:, h * (HD + 1) : h * (HD + 1) + HD] = blk
        va_all[:, h * (HD + 1) + HD] = np.where(maskc, inv32, np.zeros((), F8))
    # padded (invalid) rows must not contribute: their weight from the
    # e4m3-Schraudolph path can be a -0.0 bit pattern, which is still zero,
    # and va rows are zeroed, so context and norm are unaffected.

    ebT8 = (np.ascontiguousarray(
        relative_position_bias[0].transpose(0, 2, 1)) * 64.0).astype(F8)
    eb_c = ebT8[:, idx_pad, :]  # [H, TNV, SQ] fp8
    # pre-slab per core: for each (qc, b) iteration a contiguous
    # [128, NT, HPC, 512] block, flattened to [128, total] columns
    NQC = SQ // 512
    ebs_cols = NQC * TNT * H // NCORES * 512 // HPC * HPC  # = NQC*TNT*HPC*512 per.. wait
    ebs_all = []
    for c in range(NCORES):
        ecc = eb_c[c * HPC : (c + 1) * HPC]  # [HPC, TNV, SQ]
        blocks = []
        for qc in range(NQC):
            for b in range(B):
                nt = nvts[b]
                sub = ecc[:, snvt[b] * 128 : snvt[b + 1] * 128,
                          qc * 512 : (qc + 1) * 512]          # [HPC, nt*128, 512]
                sub = sub.reshape(HPC, nt, 128, 512).transpose(2, 1, 0, 3)
                blocks.append(sub.reshape(128, nt * HPC * 512))
        ebs_all.append(np.ascontiguousarray(np.concatenate(blocks, axis=1)))

    i2_host = np.zeros((128, 256), dtype=F8)
    i2_host[:, 0:128] = np.eye(128, dtype=np.float32).astype(F8)

    in2 = []
    for c in range(NCORES):
        rs = slice(c * 128, (c + 1) * 128)
        in2.append({
            "qT": np.ascontiguousarray(qT_full[rs, :]),
            "kT": np.ascontiguousarray(kT_c[rs, :]),
            "va": np.ascontiguousarray(
                va_all[:, c * HPC * (HD + 1) : (c + 1) * HPC * (HD + 1)]
            ),
            "ebs": ebs_all[c],
            "i2": i2_host,
        })
    r2 = _run(_get_program(("p2",) + nvts, build_phase2, nvts), in2)

    # ctx_o[c] is [128 q-part, t, 128 d] for d-block c -> assemble ctxT [D, B*SQ]
    ctxT_full = np.empty((D, B * SQ), dtype=F8)
    for c in range(NCORES):
        blk = r2.results[c]["ctx_o"].reshape(128, B * SQ // 128, 128)
        ctxT_full[c * 128 : (c + 1) * 128, :] = (
            blk.transpose(2, 1, 0).reshape(128, B * SQ)
        )

    # ---------------- phase 3 ----------------
    woT8 = wT8(Wo, 64.0)
    q2d = query.reshape(-1, D)
    resid_h = (q2d + np.asarray(bo, np.float32)[None, :]).astype(BF16)
    trivial = (not np.any(np.asarray(ln_beta))) and np.all(
        np.asarray(ln_gamma, np.float32) == 1.0
    )
    in3 = []
    for c in range(NCORES):
        sl = slice(c * RPC, (c + 1) * RPC)
        d = {
            "ctxT": np.ascontiguousarray(ctxT_full[:, sl]),
            "woT": woT8,
            "resid": np.ascontiguousarray(resid_h[sl, :]),
        }
        if not trivial:
            d["gammab"] = np.ascontiguousarray(
                np.broadcast_to(np.asarray(ln_gamma, np.float32)[None, :], (128, D))
            )
            d["betab"] = np.ascontiguousarray(
                np.broadcast_to(np.asarray(ln_beta, np.float32)[None, :], (128, D))
            )
        in3.append(d)
    r3 = _run(_get_program(("p3", trivial), build_phase3, trivial), in3)

    out = np.empty((B * SQ, D), dtype=np.float32)
    for c in range(NCORES):
        out[c * RPC : (c + 1) * RPC, :] = r3.results[c]["out_o"].astype(np.float32)

    if _collect_results is not None:
        _collect_results.extend([r1, r2, r3])
    return out.reshape(B, SQ, D)

